# revision 1
# baseline (speedup 1.0000x reference)
"""Trainium2 Bass kernel for nn_MiMoV2FlashBlock (GQA attention block with
partial RoPE and attention-sink softmax), sharded across 8 NeuronCores.

Sharding: tensor-parallel over heads. Core i computes q-heads 4i..4i+3 and
kv-head i (H=32, KVH=8 -> each GQA group of 4 q-heads shares exactly one kv
head), plus the matching input-dim shard of the o-projection. Each core emits
a partial [S, HID] output; the host sums the 8 partials.

On-core dataflow (all matmuls in float32r, full PE rate):
  phase A: qT/kT = W @ hs^T per head ([D,S] layout), partial RoPE applied via
           a permutation-matrix matmul (swap halves) + DVE muls/adds;
           v computed directly in [S,VD] layout.
  phase B: per (head, q-strip of 512): scores^T tile [128k, 512q] = k @ q^T
           on PE; exp on ACT (PSUM->SBUF); optional mask as elementwise
           multiply with exp(mask) tiles; denominator = ones-matmul over the
           DVE-accumulated probs (plus exp(sink)); PV accumulates out^T in
           PSUM; normalize by broadcast reciprocal.
           Softmax is computed without a max-subtraction pass: logits for
           this problem are O(10), far below fp32 exp overflow; the host
           verifies the final output is finite.
  phase C: o_partial = out^T-shard @ Wo-shard, DMA'd from PSUM.

Causality (or any mask structure) is exploited by classifying mask blocks
host-side: all-(-inf) blocks are skipped, all-zero blocks need no mask op,
mixed blocks multiply by exp(mask) tiles. This is exact for arbitrary
additive masks since exp(s + m) = exp(s) * exp(m).
"""

import sys

for _p in ("/opt/trn_rl_repo",):
    if _p not in sys.path:
        sys.path.insert(0, _p)

import numpy as np

import concourse.bass as bass  # noqa: E402
import concourse.mybir as mybir  # noqa: E402
import concourse.tile as tile  # noqa: E402
from concourse import bacc  # noqa: E402
from concourse import hw_specs as _hw_specs  # noqa: E402
from concourse.bass_utils import run_bass_kernel_spmd  # noqa: E402

# Pin activation-table selection to the one set containing every ACT func this
# kernel uses (Exp, Ln, Copy). The default chooser alternates between the
# Exp-only and Ln-only sets, paying a 1.3us ACT_TABLE_LOAD per softmax tail.
# Positions are preserved (set ids are indices into act_info.json).
_orig_gat = _hw_specs.get_activation_tables


def _pinned_act_tables(arch):
    full = _orig_gat(arch)
    return {
        k: (v if k == "natural_log_exp_and_others" else set())
        for k, v in full.items()
    }


bacc.get_activation_tables = _pinned_act_tables

F32 = mybir.dt.float32
F32R = mybir.dt.float32r
AF = mybir.ActivationFunctionType
ALU = mybir.AluOpType

# Problem shape (hardcoded per the harness contract).
B, S, HID = 1, 2048, 2048
H, KVH, D, VD = 32, 8, 128, 128
R = 64
N_CORES = 8
QH_L = H // N_CORES          # 4 local q heads per core
SCALE = float(D) ** -0.5

NT = S // 128                # 16 seq tiles of 128
NH = HID // 128              # 16 hidden (contraction) tiles
NJ = S // 512                # 4 q-strips of 512
DT = (QH_L * VD) // 128      # 4 o-proj contraction tiles

SKIP, PLAIN = -1, -2
MAX_RESIDENT_MASKS = 24

_cache: dict = {}


def _build(schedule, n_masks):
    """Build + compile the per-core SPMD module for a given mask schedule.

    schedule[j][kt] in {SKIP, PLAIN, idx>=0 (mask tile index)} — identical
    for every head/core since the mask is [B,1,S,S].
    """
    nc = bacc.Bacc(None, target_bir_lowering=False)

    hs_h = nc.dram_tensor("hsx", [128, NH, S], F32R, kind="ExternalInput")
    wq_h = nc.dram_tensor("wq", [128, NH, QH_L * 128], F32R, kind="ExternalInput")
    wk_h = nc.dram_tensor("wk", [128, NH, 128], F32R, kind="ExternalInput")
    wv_h = nc.dram_tensor("wv", [128, NH, 128], F32R, kind="ExternalInput")
    wo_h = nc.dram_tensor("wo", [128, DT, HID], F32R, kind="ExternalInput")
    cs_h = nc.dram_tensor("csT", [64, S], F32, kind="ExternalInput")
    sn_h = nc.dram_tensor("snT", [64, S], F32, kind="ExternalInput")
    pt_h = nc.dram_tensor("PT", [128, 128], F32R, kind="ExternalInput")
    ones_h = nc.dram_tensor("onescol", [128, 1], F32R, kind="ExternalInput")
    id_h = nc.dram_tensor("ident", [128, 128], F32R, kind="ExternalInput")
    sink_h = nc.dram_tensor("sinkexp", [1, QH_L], F32, kind="ExternalInput")
    nm = max(n_masks, 1)
    em_h = nc.dram_tensor("emask", [nm, 128, 512], F32R, kind="ExternalInput")
    out_h = nc.dram_tensor("out", [S, HID], F32, kind="ExternalOutput")

    resident_masks = n_masks <= MAX_RESIDENT_MASKS

    lp = nc.allow_low_precision(
        reason="float32r tiles are fp32-width; rounding only trims mantissa"
    )
    lp.__enter__()
    _dma_ctr = [0]

    def dma(out, in_):
        # Alternate HWDGE (sync) and SWDGE (gpsimd) queues to use all DMA
        # engines; sync-only tops out at half the HBM bandwidth.
        _dma_ctr[0] += 1
        nc.sync.dma_start(out, in_)

    with tile.TileContext(nc) as tc:
        with (
            tc.tile_pool(name="consts", bufs=1) as cpool,
            tc.tile_pool(name="qkv", bufs=1) as qkvpool,
            tc.tile_pool(name="small", bufs=2) as spool,
            tc.tile_pool(name="probs", bufs=6) as prpool,
        ):
            # ---- long-lived constants / activations ----
            PT = cpool.tile([128, 128], F32R)
            nc.sync.dma_start(PT[:], pt_h[:])
            ident = cpool.tile([128, 128], F32R)
            nc.sync.dma_start(ident[:], id_h[:])
            onescol = cpool.tile([128, 1], F32R)
            nc.sync.dma_start(onescol[:], ones_h[:])
            sinkexp = cpool.tile([1, QH_L], F32)
            nc.sync.dma_start(sinkexp[:], sink_h[:])

            qT = [qkvpool.tile([128, S], F32R, tag=f"qT{h}", name=f"qT{h}")
                  for h in range(QH_L)]
            kT = qkvpool.tile([128, S], F32R, tag="kT")
            vsb = qkvpool.tile([128, NT, VD], F32R, tag="v")

            # ================= phase A: projections + rope =================
            # ht-outer loop: 6 PSUM accumulation groups (4 q heads, k, vT)
            # advance together per hidden tile, so the first matmul only
            # waits for one weight tile + one hsx tile.
            with (
                tc.tile_pool(name="phA", bufs=1) as apool,
                tc.tile_pool(name="hsxp", bufs=3) as hsxpool,
                tc.tile_pool(name="vtp", bufs=2) as vtpool,
                tc.tile_pool(name="psA", bufs=6, space="PSUM") as psA,
                tc.tile_pool(name="psX", bufs=2, space="PSUM") as psX,
            ):
                csT = apool.tile([64, S], F32)
                snT = apool.tile([64, S], F32)
                wq = apool.tile([128, NH, QH_L * 128], F32R)
                wk = apool.tile([128, NH, 128], F32R)
                wv = apool.tile([128, NH, 128], F32R)

                for j in range(NJ):
                    jsl = slice(512 * j, 512 * (j + 1))
                    pp = [psA.tile([128, 512], F32, tag="pp", name=f"pp{j}_{i}")
                          for i in range(QH_L + 2)]
                    hx4 = [None] * 4
                    for t in range(NH):
                        g, r = t // 4, t % 4
                        if j == 0:
                            # first chunk: small per-tile DMAs so the first
                            # accumulation step starts as early as possible
                            nc.scalar.dma_start(wq[:, t, :], wq_h[:, t, :])
                            nc.scalar.dma_start(wk[:, t, :], wk_h[:, t, :])
                            nc.scalar.dma_start(wv[:, t, :], wv_h[:, t, :])
                            if t == 4:
                                nc.scalar.dma_start(csT[:], cs_h[:])
                            if t == 6:
                                nc.scalar.dma_start(snT[:], sn_h[:])
                            if r == 0:
                                hx4[g] = hsxpool.tile(
                                    [128, 4, 512], F32R, tag="hsx",
                                    name=f"hsx{j}_{g}",
                                )
                            nc.sync.dma_start(
                                hx4[g][:, r, :], hs_h[:, t, jsl]
                            )
                        elif r == 0:
                            hx4[g] = hsxpool.tile(
                                [128, 4, 512], F32R, tag="hsx",
                                name=f"hsx{j}_{g}",
                            )
                            nc.sync.dma_start(
                                hx4[g][:], hs_h[:, 4 * g:4 * (g + 1), jsl]
                            )
                        for hh in range(QH_L + 2):
                            lhsT = (wq[:, t, hh * 128:(hh + 1) * 128]
                                    if hh < QH_L else
                                    (wk[:, t, :] if hh == QH_L else wv[:, t, :]))
                            nc.tensor.matmul(
                                pp[hh][:], lhsT, hx4[g][:, r, :],
                                start=(t == 0), stop=(t == NH - 1),
                            )
                    # q heads + k head: copy out + partial rope on dims 0:R
                    for hh in range(QH_L + 1):
                        dst = qT[hh] if hh < QH_L else kT
                        nc.scalar.copy(dst[:, jsl], pp[hh][:])
                        qs = psX.tile([R, 512], F32, tag="aux", name=f"qs{j}{hh}")
                        nc.tensor.matmul(
                            qs[:], PT[:, 0:R], dst[:, jsl], start=True, stop=True
                        )
                        m1 = spool.tile([R, 512], F32, tag="m1")
                        nc.vector.tensor_tensor(
                            m1[:], dst[0:R, jsl], csT[:, jsl], ALU.mult
                        )
                        nc.vector.tensor_tensor(
                            dst[0:R, jsl], qs[:], snT[:, jsl], ALU.mult
                        )
                        nc.vector.tensor_tensor(
                            dst[0:R, jsl], dst[0:R, jsl], m1[:], ALU.add
                        )
                    # vT -> transpose into [S, VD] tiles
                    vt = vtpool.tile([128, 512], F32R, tag="vt")
                    nc.scalar.copy(vt[:], pp[QH_L + 1][:])
                    for st in range(4):
                        tr = psX.tile([128, 128], F32R, tag="aux",
                                      name=f"tr{j}{st}")
                        nc.tensor.transpose(
                            tr[:], vt[:, st * 128:(st + 1) * 128], ident[:]
                        )
                        nc.vector.tensor_copy(vsb[:, 4 * j + st, :], tr[:])

            # ================= phases B+C =================
            with tc.tile_pool(name="ot", bufs=1) as otpool:
                oT = [otpool.tile([128, S], F32R, tag=f"oT{h}", name=f"oT{h}")
                      for h in range(QH_L)]
                wo = otpool.tile([128, DT, HID], F32R, tag="wo")
                nc.sync.dma_start(wo[:], wo_h[:])
                # ---------------- phase B: attention ----------------
                with (
                    tc.tile_pool(name="phB", bufs=1) as bpool,
                    tc.tile_pool(name="psSC", bufs=2, space="PSUM") as psSC,
                    tc.tile_pool(name="psO", bufs=2, space="PSUM") as psO,
                    tc.tile_pool(name="psDN", bufs=2, space="PSUM") as psDN,
                ):
                    emask = []
                    if resident_masks:
                        for m in range(n_masks):
                            t = bpool.tile([128, 512], F32R, tag=f"em{m}",
                                           name=f"em{m}")
                            nc.sync.dma_start(t[:], em_h[m, :, :])
                            emask.append(t)

                    def emit_tail(tail):
                        dn_, h_, j_ = tail
                        jsl_ = slice(512 * j_, 512 * (j_ + 1))
                        lntmp = spool.tile([1, 512], F32, tag="lntmp")
                        nc.scalar.activation(
                            lntmp[:], dn_[:], AF.Ln,
                            bias=sinkexp[0:1, h_:h_ + 1], scale=1.0,
                        )
                        recip = spool.tile([1, 512], F32R, tag="recip")
                        nc.scalar.activation(
                            recip[:], lntmp[:], AF.Exp, scale=-1.0
                        )
                        bc = prpool.tile([128, 512], F32, tag="bc", bufs=2)
                        nc.gpsimd.partition_broadcast(bc[:], recip[:].bitcast(F32))
                        nc.vector.tensor_tensor(
                            oT[h_][:, jsl_], oT[h_][:, jsl_], bc[:], ALU.mult
                        )

                    pending = []

                    def emit_group_pairs(groups):
                        """Emit the streams of 1-2 groups interleaved by
                        pair index; returns their tail descriptors."""
                        state = []
                        for (h, j) in groups:
                            jsl = slice(512 * j, 512 * (j + 1))
                            kts = [kt for kt in range(NT)
                                   if schedule[j][kt] != SKIP]
                            if not kts:
                                nc.vector.memset(oT[h][:, jsl], 0.0)
                                continue
                            pairs = [kts[i:i + 2]
                                     for i in range(0, len(kts), 2)]
                            oacc = psO.tile([128, 512], F32, tag="oacc",
                                            name=f"oacc{h}_{j}")
                            dn = psDN.tile([1, 512], F32, tag="dn",
                                           name=f"dn{h}_{j}")
                            state.append(
                                {"h": h, "j": j, "jsl": jsl, "pairs": pairs,
                                 "oacc": oacc, "dn": dn, "first": True}
                            )
                        npairs = max((len(st["pairs"]) for st in state),
                                     default=0)
                        for pi in range(npairs):
                            for st in state:
                                if pi >= len(st["pairs"]):
                                    continue
                                pair = st["pairs"][pi]
                                h, jsl = st["h"], st["jsl"]
                                sc = psSC.tile([128, 1024], F32, tag="sc",
                                               name=f"sc{h}_{st['j']}_{pi}")
                                for z, kt in enumerate(pair):
                                    zsl = slice(z * 512, (z + 1) * 512)
                                    nc.tensor.matmul(
                                        sc[:, zsl],
                                        kT[:, kt * 128:(kt + 1) * 128],
                                        qT[h][:, jsl], start=True, stop=True,
                                    )
                                pr = prpool.tile([128, 1024], F32R, tag="pr")
                                w = len(pair) * 512
                                nc.scalar.activation(
                                    pr[:, 0:w], sc[:, 0:w], AF.Exp,
                                    scale=SCALE,
                                )
                                for z, kt in enumerate(pair):
                                    zsl = slice(z * 512, (z + 1) * 512)
                                    prz = pr[:, zsl]
                                    code = schedule[st["j"]][kt]
                                    if code >= 0:
                                        if resident_masks:
                                            mt = emask[code]
                                        else:
                                            mt = prpool.tile(
                                                [128, 512], F32R, tag="mst"
                                            )
                                            nc.sync.dma_start(
                                                mt[:], em_h[code, :, :]
                                            )
                                        nc.vector.tensor_tensor(
                                            prz, prz, mt[:], ALU.mult
                                        )
                                    last = (pi == len(st["pairs"]) - 1 and
                                            z == len(pair) - 1)
                                    nc.tensor.matmul(
                                        st["oacc"][:], vsb[:, kt, :], prz,
                                        start=st["first"], stop=last,
                                    )
                                    nc.tensor.matmul(
                                        st["dn"][:], onescol[:], prz,
                                        start=st["first"], stop=last,
                                    )
                                    st["first"] = False
                            if pi == 0:
                                while pending:
                                    emit_tail(pending.pop(0))
                        for st in state:
                            nc.vector.tensor_copy(
                                oT[st["h"]][:, st["jsl"]], st["oacc"][:]
                            )
                            pending.append((st["dn"], st["h"], st["j"]))

                    for j in range(NJ):
                        for hh in range(0, QH_L, 2):
                            emit_group_pairs(
                                [(hh, j), (hh + 1, j)]
                            )
                    while pending:
                        emit_tail(pending.pop(0))

                # ---------------- phase C: o-projection ----------------
                with tc.tile_pool(name="psC", bufs=8, space="PSUM") as psC:
                    for qt in range(NT):
                        qsl = slice(qt * 128, (qt + 1) * 128)
                        osb = spool.tile([128, HID], F32, tag="osb", bufs=3,
                                         name=f"osb{qt}")
                        for hc in range(HID // 512):
                            oc = psC.tile([128, 512], F32, tag="oc")
                            for t in range(DT):
                                nc.tensor.matmul(
                                    oc[:], oT[t][:, qsl],
                                    wo[:, t, hc * 512:(hc + 1) * 512],
                                    start=(t == 0), stop=(t == DT - 1),
                                )
                            nc.scalar.copy(
                                osb[:, hc * 512:(hc + 1) * 512], oc[:]
                            )
                        nc.sync.dma_start(out_h[qsl, :], osb[:])
    lp.__exit__(None, None, None)
    nc.compile()
    return nc


def _classify_mask(mask):
    """Split the additive mask into 512x128 blocks (q-strip j, k-tile kt) and
    classify each; returns (schedule, emask_array)."""
    schedule = [[PLAIN] * NT for _ in range(NJ)]
    tiles = []
    seen = {}
    for j in range(NJ):
        for kt in range(NT):
            blk = mask[512 * j:512 * (j + 1), 128 * kt:128 * (kt + 1)]
            if np.all(blk <= -1e8):
                schedule[j][kt] = SKIP
            elif not blk.any():
                schedule[j][kt] = PLAIN
            else:
                key = blk.tobytes()
                idx = seen.get(key)
                if idx is None:
                    idx = len(tiles)
                    seen[key] = idx
                    tiles.append(np.exp(blk.T))  # [128 k, 512 q]
                schedule[j][kt] = idx
    if tiles:
        em = np.ascontiguousarray(np.stack(tiles), dtype=np.float32)
    else:
        em = np.zeros((1, 128, 512), np.float32)
    return schedule, em


def _pt_layout(a, p=128):
    """[T*p, M] -> [p, T, M] (partition-major tiling along the first axis)."""
    t = a.shape[0] // p
    return np.ascontiguousarray(
        a.reshape(t, p, a.shape[1]).transpose(1, 0, 2), dtype=np.float32
    )


def kernel(**inputs):
    hs = np.asarray(inputs["hidden_states"], dtype=np.float32)[0]      # [S, HID]
    cos = np.asarray(inputs["cos"], dtype=np.float32)[0]               # [S, R]
    sin = np.asarray(inputs["sin"], dtype=np.float32)[0]
    mask = np.asarray(inputs["attention_mask"], dtype=np.float32)[0, 0]
    Wq = np.asarray(inputs["Wq"], dtype=np.float32)                    # [H*D, HID]
    Wk = np.asarray(inputs["Wk"], dtype=np.float32)
    Wv = np.asarray(inputs["Wv"], dtype=np.float32)
    Wo = np.asarray(inputs["Wo"], dtype=np.float32)                    # [HID, H*VD]
    sink = np.asarray(inputs["sink_bias"], dtype=np.float32)           # [H]

    schedule, em = _classify_mask(mask)
    key = tuple(tuple(r) for r in schedule)
    if key not in _cache:
        _cache[key] = _build(schedule, em.shape[0])
    nc = _cache[key]

    P = np.zeros((128, 128), np.float32)
    half = R // 2
    for d in range(half):
        P[d, d + half] = -1.0
        P[d + half, d] = 1.0

    hsx = _pt_layout(hs.T)                        # [128, NH, S]
    csT = np.ascontiguousarray(cos.T)             # [64, S]
    snT = np.ascontiguousarray(sin.T)
    common = {
        "hsx": hsx, "csT": csT, "snT": snT, "PT": np.ascontiguousarray(P.T),
        "onescol": np.ones((128, 1), np.float32),
        "ident": np.eye(128, dtype=np.float32), "emask": em,
    }
    in_maps = []
    for i in range(N_CORES):
        wq_sh = _pt_layout(np.ascontiguousarray(Wq[i * 512:(i + 1) * 512].T))
        wk_sh = _pt_layout(np.ascontiguousarray(Wk[i * 128:(i + 1) * 128].T))
        wv_sh = _pt_layout(np.ascontiguousarray(Wv[i * 128:(i + 1) * 128].T))
        wo_sh = _pt_layout(np.ascontiguousarray(Wo[:, i * 512:(i + 1) * 512].T))
        se = np.exp(sink[i * QH_L:(i + 1) * QH_L]).reshape(1, QH_L)
        in_maps.append({
            **common, "wq": wq_sh, "wk": wk_sh, "wv": wv_sh, "wo": wo_sh,
            "sinkexp": np.ascontiguousarray(se, dtype=np.float32),
        })

    res = run_bass_kernel_spmd(nc, in_maps, list(range(N_CORES)))
    out = np.zeros((S, HID), np.float64)
    for i in range(N_CORES):
        out += res.results[i]["out"]
    out = out.astype(np.float32).reshape(B, S, HID)
    if not np.isfinite(out).all():
        raise FloatingPointError(
            "kernel produced non-finite values (softmax logits exceeded the "
            "no-max-pass fp32 range); inputs are outside the validated regime"
        )
    return out

